# revision 36
# baseline (speedup 1.0000x reference)
# kernel.py — Bass/Trainium2 kernel for nn_GCNBaseNet (gnn_message_passing)
#
# Sharding: data-parallel over graphs (8 cores x 32 graphs, replicated weights).
#
# Math restructuring (per layer, per graph):
#   reference:  h' = relu(concat_r(A_r h W_r + b_r) @ Wi1 + bi1) @ Wi2 + bi2
#   using concat_r(m_r) @ Wi1 = sum_r m_r @ Wi1_r  and A_r(h W_r) Wi1_r =
#   A_r (h (W_r Wi1_r)):
#       h' = relu(sum_r A_r (h @ Wfused_{l,r}) + c_l) @ Wi2 + bi2
#   with Wfused_{l,r} = W_{l,r} @ Wi1_r (computed on device) and
#   c_l = bi1 + sum_r b_{l,r} @ Wi1_r.
#
# Layout: activations are feature-major (hT: [D, nodes]) the whole way.
#
# A^T build (per pair of graphs, PE one-hot scatter):
#   - edge data loaded in NATIVE layout (2 contiguous DMAs), transposed
#     on-device via PE (int values exact in f32), converted to bf16.
#   - indices/weights duplicated into adjacent pairs (gpsimd copies) so the
#     one-hot is_equal and the x-w multiply run on DVE with every operand
#     2-byte stride-1 (DVE 2x fast mode) despite the broadcast.
#   - scatter via PE matmuls (one-hot^T @ one-hot), two graphs packed in PE
#     column groups; an extra all-w lhsT column yields the degree row free.
#   - per-pair: degree row -> ACT Rsqrt -> dis outer products (K=1 PE
#     matmuls) -> gpsimd elementwise (scat+I) * (dis x dis). No global
#     barrier; pairs pipeline across DVE/PE/ACT/gpsimd.
#
# Final FC: z1^T = sum_k hT3-slice^T @ Wf1-chunk with Wf1 in its native
# layout (1KB packets), bias added via a K=1 ones-row matmul, Wf1 prefetched
# in two big batches that hide under the A-build and layer phases.
import numpy as np

G, N, F, D, R, E, L = 256, 60, 128, 256, 4, 512, 3
NCORES = 8
GC = G // NCORES  # graphs per core
C = E // 128      # edge chunks per (g, r)

_CACHE = {}


def _build(gc, enable_asserts=False):
    """Builds the full Bass module for `gc` graphs on one core."""
    from contextlib import ExitStack

    import concourse.mybir as mybir
    import concourse.tile as tile
    from concourse import bacc
    from concourse.masks import make_identity

    dt = mybir.dt
    f32, f32r, bf16, i32 = dt.float32, dt.float32r, dt.bfloat16, dt.int32
    AF = mybir.ActivationFunctionType
    OP = mybir.AluOpType

    npair = gc // 2
    nn = gc * N                      # nodes per core
    nt = (nn + 127) // 128           # x row tiles
    NKC = (N * D) // 128             # wf1 K chunks (120)

    nc = bacc.Bacc(
        "TRN2",
        target_bir_lowering=False,
        debug=False,
        enable_asserts=enable_asserts,
        num_devices=NCORES,
    )

    # ---- DRAM tensors -----------------------------------------------------
    x_d = nc.dram_tensor("x", [nn, F], f32, kind="ExternalInput").ap()
    ei_d = nc.dram_tensor("ei", [gc * R * 2, E], i32, kind="ExternalInput").ap()
    ew_d = nc.dram_tensor("ew", [gc * R, E], f32, kind="ExternalInput").ap()
    w0_d = nc.dram_tensor("w0", [R, F, D], f32, kind="ExternalInput").ap()
    wg_d = nc.dram_tensor("wg", [L - 1, R, D, D], f32, kind="ExternalInput").ap()
    b0_d = nc.dram_tensor("b0", [R * D], f32, kind="ExternalInput").ap()
    bg_d = nc.dram_tensor("bg", [L - 1, R * D], f32, kind="ExternalInput").ap()
    wi1_d = nc.dram_tensor("wi1", [R * D, D], f32, kind="ExternalInput").ap()
    bi1_d = nc.dram_tensor("bi1", [D], f32, kind="ExternalInput").ap()
    wi2_d = nc.dram_tensor("wi2", [D, D], f32, kind="ExternalInput").ap()
    bi2_d = nc.dram_tensor("bi2", [D], f32, kind="ExternalInput").ap()
    wf1_d = nc.dram_tensor("wf1", [N * D, D], f32, kind="ExternalInput").ap()
    bf1_d = nc.dram_tensor("bf1", [D], f32, kind="ExternalInput").ap()
    wf2_d = nc.dram_tensor("wf2", [D, D], f32, kind="ExternalInput").ap()
    bf2_d = nc.dram_tensor("bf2", [D], f32, kind="ExternalInput").ap()
    wf3_d = nc.dram_tensor("wf3", [D, 2], f32, kind="ExternalInput").ap()
    bf3_d = nc.dram_tensor("bf3", [2], f32, kind="ExternalInput").ap()
    out_d = nc.dram_tensor("out", [gc, 2], f32, kind="ExternalOutput").ap()

    with tile.TileContext(nc) as tc:
        with ExitStack() as top:
            persist = top.enter_context(tc.tile_pool(name="persist", bufs=1))

            # ---- constants ----
            ident = persist.tile([128, 128], f32)
            make_identity(nc, ident[:])
            iota62 = persist.tile([128, 62], bf16)
            i60 = persist.tile([60, 60], bf16)
            nc.gpsimd.memset(i60[:], 0.0)
            nc.gpsimd.affine_select(
                out=i60[:], in_=i60[:], compare_op=OP.not_equal, fill=1.0,
                base=0, pattern=[[-1, 60]], channel_multiplier=1,
            )
            selfT = persist.tile([60, 61], bf16)
            nc.gpsimd.memset(selfT[:], 0.0)
            nc.gpsimd.affine_select(
                out=selfT[:, 0:60], in_=selfT[:, 0:60], compare_op=OP.not_equal,
                fill=1.0, base=0, pattern=[[-1, 60]], channel_multiplier=1,
            )
            nc.gpsimd.memset(selfT[:, 60:61], 1.0)
            ones1 = persist.tile([1, gc], f32)
            nc.gpsimd.memset(ones1[:], 1.0)


            # feature-major bias vectors [128, 2] (chunk-major)
            def load_fm(name, ap):
                t = persist.tile([128, 2], f32, name=name, tag=name)
                nc.sync.dma_start(out=t[:], in_=ap.rearrange("(m p) -> p m", p=128))
                return t

            bi2_fm = load_fm("bi2_fm", bi2_d)
            bf2_fm = load_fm("bf2_fm", bf2_d)
            bf1_row = persist.tile([1, D], f32)
            nc.sync.dma_start(out=bf1_row[:], in_=bf1_d[None, :])
            bf3_fm = persist.tile([2, 1], f32)
            nc.sync.dma_start(out=bf3_fm[:], in_=bf3_d[:, None])
            bi1_fm = load_fm("bi1_fm", bi1_d)

            # persistent weights (staging buffers live in the prep pool)
            wi2r = persist.tile([128, 2, D], f32r)
            wf2b = persist.tile([128, 2, D], bf16)
            wf3b = persist.tile([128, 2, 2], bf16)
            fs0_sb = persist.tile([128, R, D], f32r)           # Wfused layer 0
            fsg_sb = persist.tile([128, L - 1, 2, R, D], f32r)  # [l, kc, r, d]
            c_sb = persist.tile([128, 2, L], f32)              # fused bias

            # AT_all: normalized block-diagonal A^T per pair [src, pair, r, tgt]
            AT_all = persist.tile([120, npair, R, 120], bf16)
            nc.gpsimd.memset(AT_all[:], 0.0)

            # ---- hT pool ----
            hT_pool = top.enter_context(tc.tile_pool(name="hT", bufs=2))
            hT = [None] * (L + 1)

            # =======================================================
            # Stage 1: input prep (weights, x transpose, edge prep)
            # =======================================================
            with ExitStack() as prep:
                wld = prep.enter_context(tc.tile_pool(name="wld", bufs=1))
                wps = prep.enter_context(
                    tc.tile_pool(name="wps", bufs=2, space="PSUM")
                )

                # constants / small weights staged here, cast into persist
                iota_i = wld.tile([128, 62], i32)
                nc.gpsimd.iota(
                    iota_i[:], pattern=[[1, 62]], base=0, channel_multiplier=0
                )
                nc.vector.tensor_copy(iota62[:], iota_i[:])
                wi2_sb = wld.tile([128, 2, D], f32)
                nc.sync.dma_start(
                    out=wi2_sb[:], in_=wi2_d.rearrange("(c p) d -> p c d", p=128)
                )
                nc.vector.tensor_copy(wi2r[:], wi2_sb[:])
                wf2_sb = wld.tile([128, 2, D], f32)
                nc.sync.dma_start(
                    out=wf2_sb[:], in_=wf2_d.rearrange("(c p) d -> p c d", p=128)
                )
                nc.vector.tensor_copy(wf2b[:], wf2_sb[:])
                wf3_sb = wld.tile([128, 2, 2], f32)
                nc.sync.dma_start(
                    out=wf3_sb[:], in_=wf3_d.rearrange("(c p) j -> p c j", p=128)
                )
                nc.vector.tensor_copy(wf3b[:], wf3_sb[:])

                # -- edge data: native-layout DMAs --
                eiN = wld.tile([128, 2, E], i32)
                nc.sync.dma_start(
                    out=eiN[:], in_=ei_d.rearrange("(t p) e -> p t e", p=128)
                )
                ewN = wld.tile([128, E], f32)
                nc.sync.dma_start(out=ewN[:], in_=ew_d[:, :])
                # i32 -> f32 value-convert in place (bit widths match)
                eiNf = eiN[:].bitcast(f32)
                nc.vector.tensor_copy(eiNf, eiN[:])

                # PE transposes: eiTf[p, c, t, row], ewTf[p, c, row]
                eiTf = wld.tile([128, C, 2, 128], f32)
                ewTf = wld.tile([128, C, 128], f32)
                for t in range(2):
                    for c in range(C):
                        tp = wps.tile([128, 128], f32, tag="tp")
                        nc.tensor.transpose(
                            tp[:], eiNf[:, t, 128 * c:128 * (c + 1)], ident[:]
                        )
                        nc.scalar.copy(eiTf[:, c, t, :], tp[:])
                del eiNf
                for c in range(C):
                    tp = wps.tile([128, 128], f32, tag="tp")
                    nc.tensor.transpose(
                        tp[:], ewN[:, 128 * c:128 * (c + 1)], ident[:]
                    )
                    nc.scalar.copy(ewTf[:, c, :], tp[:])

                # duplicated-pair index/weight tables (enable DVE 2x mode):
                # eidx2[p, g, r, two, c, dup], ew2[p, g, r, c, dup]
                eidx2 = persist.tile([128, gc, R, 2, C, 2], bf16)
                ew2 = persist.tile([128, gc, R, C, 2], bf16)
                for t in range(2):
                    src = eiTf[:, :, t, :].rearrange(
                        "p c (g r two) -> p g r two c", g=16, r=R, two=2
                    )
                    for dup in range(2):
                        nc.vector.tensor_copy(
                            out=eidx2[:, 16 * t:16 * (t + 1), :, :, :, dup],
                            in_=src,
                        )
                for dup in range(2):
                    nc.vector.tensor_copy(
                        out=ew2[:, :, :, :, dup],
                        in_=ewTf[:].rearrange(
                            "p c (g r) -> p g r c", g=gc, r=R
                        ),
                    )

                # -- x load (one DMA) + transpose --> hT[0] --
                hT[0] = hT_pool.tile([128, 2, nn], f32r, tag="hT", name="hT0")
                xt = wld.tile([128, nt, 128], f32)
                nc.sync.dma_start(
                    out=xt[:],
                    in_=x_d.rearrange("(t p) f -> p t f", p=128),
                )
                for t in range(nt):
                    tp = wps.tile([128, 128], f32, tag="tp")
                    nc.tensor.transpose(tp[:], xt[:, t, :], ident[:])
                    nc.scalar.copy(hT[0][:, 0, 128 * t:128 * (t + 1)], tp[:])

                # -- weight prep --
                wi1_sb = wld.tile([128, 2 * R, D], f32)
                nc.scalar.dma_start(
                    out=wi1_sb[:], in_=wi1_d.rearrange("(c p) d -> p c d", p=128)
                )
                w0_sb = wld.tile([128, R, D], f32)
                nc.sync.dma_start(
                    out=w0_sb[:], in_=w0_d.rearrange("r p d -> p r d")
                )
                wg_sb = wld.tile([128, L - 1, R, 2, D], f32)
                nc.scalar.dma_start(
                    out=wg_sb[:],
                    in_=wg_d.rearrange("l r (c p) d -> p l r c d", p=128),
                )
                wi1r = wld.tile([128, 2 * R, D], f32r)
                nc.vector.tensor_copy(wi1r[:], wi1_sb[:])
                w0T_sb = wld.tile([128, R, 2, 128], f32r)
                wgT_sb = wld.tile([128, L - 1, R, 2, 2, 128], f32r)

                for r in range(R):
                    for j in range(2):
                        tp = wps.tile([128, 128], f32, tag="tp")
                        nc.tensor.transpose(
                            tp[:], w0_sb[:, r, 128 * j:128 * (j + 1)], ident[:]
                        )
                        nc.scalar.copy(w0T_sb[:, r, j, :], tp[:])
                for l in range(L - 1):
                    for r in range(R):
                        for ja in range(2):
                            for fb in range(2):
                                tp = wps.tile([128, 128], f32, tag="tp")
                                nc.tensor.transpose(
                                    tp[:],
                                    wg_sb[:, l, r, fb, 128 * ja:128 * (ja + 1)],
                                    ident[:],
                                )
                                nc.scalar.copy(wgT_sb[:, l, r, ja, fb, :], tp[:])

                # Wfused = (W^T).T @ Wi1_r  (K = inner D, accumulated)
                for r in range(R):
                    fpp = wps.tile([128, D], f32, tag="fp")
                    for jc in range(2):
                        nc.tensor.matmul(
                            fpp[:],
                            lhsT=w0T_sb[:, r, jc, :],
                            rhs=wi1r[:, 2 * r + jc, :],
                            start=(jc == 0), stop=(jc == 1),
                        )
                    nc.scalar.copy(fs0_sb[:, r, :], fpp[:])
                for l in range(L - 1):
                    for r in range(R):
                        for fb in range(2):
                            fpp = wps.tile([128, D], f32, tag="fp")
                            for jc in range(2):
                                nc.tensor.matmul(
                                    fpp[:],
                                    lhsT=wgT_sb[:, l, r, jc, fb, :],
                                    rhs=wi1r[:, 2 * r + jc, :],
                                    start=(jc == 0), stop=(jc == 1),
                                )
                            nc.scalar.copy(fsg_sb[:, l, fb, r, :], fpp[:])

                # c_l = bi1 + sum_r b_lr @ Wi1_r   (feature-major [128,1] x2)
                b_sb = wld.tile([128, 2 * R, 4], f32)
                nc.gpsimd.memset(b_sb[:], 0.0)
                nc.sync.dma_start(
                    out=b_sb[:, :, 0:1],
                    in_=b0_d.rearrange("(c p) -> p c", p=128)[:, :, None],
                )
                for l in range(L - 1):
                    nc.sync.dma_start(
                        out=b_sb[:, :, l + 1:l + 2],
                        in_=bg_d[l].rearrange("(c p) -> p c", p=128)[:, :, None],
                    )
                b_sbr = wld.tile([128, 2 * R, 4], f32r)
                nc.vector.tensor_copy(b_sbr[:], b_sb[:])
                for m in range(2):
                    cp = wps.tile([128, 4], f32, tag="cp")
                    for ch in range(2 * R):
                        nc.tensor.matmul(
                            cp[:],
                            lhsT=wi1r[:, ch, 128 * m:128 * (m + 1)],
                            rhs=b_sbr[:, ch, :],
                            start=(ch == 0), stop=(ch == 2 * R - 1),
                        )
                    nc.scalar.activation(
                        c_sb[:, m, :], cp[:, 0:L], AF.Identity,
                        bias=bi1_fm[:, m:m + 1],
                    )

            # =======================================================
            # Stage 2: A-build (per-pair pipeline, no global barrier)
            # =======================================================
            with ExitStack() as ab:
                oh_pool = ab.enter_context(tc.tile_pool(name="oh", bufs=4))
                scat_sp = ab.enter_context(tc.tile_pool(name="scat_sb", bufs=3))
                stg_pool = ab.enter_context(tc.tile_pool(name="stg", bufs=3))
                ds_sp = ab.enter_context(tc.tile_pool(name="ds_sb", bufs=3))
                at1_pool = ab.enter_context(tc.tile_pool(name="at1", bufs=3))
                scat_pp = ab.enter_context(
                    tc.tile_pool(name="scat_ps", bufs=2, space="PSUM")
                )
                ds_pp = ab.enter_context(
                    tc.tile_pool(name="ds_ps", bufs=2, space="PSUM")
                )

                def emit_oh_scat(p):
                    ohs = []
                    for j in range(2):
                        g = 2 * p + j
                        oh = oh_pool.tile(
                            [128, R, 2, C, 62], bf16, tag="oh", name=f"oh{j}"
                        )
                        # one-hot compare, all operands 2-byte stride-1
                        nc.vector.tensor_tensor(
                            out=oh[:].rearrange(
                                "p r two c (s2 d2) -> p (r two c) s2 d2", s2=31
                            ),
                            in0=iota62[:].rearrange(
                                "p (s2 d2) -> p s2 d2", s2=31
                            )[:, None, :, :].to_broadcast([128, R * 2 * C, 31, 2]),
                            in1=eidx2[:, g, :, :, :, :].rearrange(
                                "p r two c d2 -> p (r two c) d2"
                            )[:, :, None, :].to_broadcast([128, R * 2 * C, 31, 2]),
                            op=OP.is_equal,
                        )
                        # x w on the src half (cols 0..59)
                        nc.vector.tensor_tensor(
                            out=oh[:, :, 0, :, 0:60].rearrange(
                                "p r c (s2 d2) -> p r c s2 d2", s2=30
                            ),
                            in0=oh[:, :, 0, :, 0:60].rearrange(
                                "p r c (s2 d2) -> p r c s2 d2", s2=30
                            ),
                            in1=ew2[:, g, :, :, :][:, :, :, None, :].to_broadcast(
                                [128, R, C, 30, 2]
                            ),
                            op=OP.mult,
                        )
                        # w column (degree row source)
                        nc.vector.tensor_copy(
                            out=oh[:, :, 0, :, 60:61],
                            in_=ew2[:, g, :, :, 0:1],
                        )
                        ohs.append(oh)

                    scat_ps = scat_pp.tile([128, R, 60], f32, tag="scat")
                    for r in range(R):
                        for c in range(C):
                            for j in range(2):
                                nc.tensor.matmul(
                                    scat_ps[64 * j:64 * j + 61, r, :],
                                    lhsT=ohs[j][:, r, 0, c, 0:61],
                                    rhs=ohs[j][:, r, 1, c, 0:60],
                                    start=(c == 0), stop=False,
                                    tile_position=(0, 64 * j),
                                )
                        for j in range(2):
                            nc.tensor.matmul(
                                scat_ps[64 * j:64 * j + 61, r, :],
                                lhsT=selfT[:],
                                rhs=i60[:],
                                start=False, stop=True,
                                tile_position=(0, 64 * j),
                            )
                    scat_sb = scat_sp.tile([128, R, 60], bf16, tag="scat_sb")
                    nc.scalar.copy(scat_sb[:], scat_ps[:])

                    # degree rows -> dis = exp(-0.5*ln(deg)) = 1/sqrt(deg)
                    # (ACT-only; avoids the slow multi-pass DVE reciprocal)
                    deg2 = stg_pool.tile([1, 2, R * 60], bf16, tag="deg")
                    for j in range(2):
                        nc.gpsimd.dma_start(
                            out=deg2[0:1, j, :],
                            in_=scat_sb[64 * j + 60:64 * j + 61, :, :],
                        )
                    ln2 = stg_pool.tile([1, 2, R * 60], f32, tag="rec")
                    nc.scalar.activation(
                        ln2[0:1, :, :], deg2[0:1, :, :], AF.Ln
                    )
                    dis2 = stg_pool.tile([1, 2, R * 60], f32, tag="dis")
                    nc.scalar.activation(
                        dis2[0:1, :, :], ln2[0:1, :, :], AF.Exp, scale=-0.5
                    )
                    return scat_sb, dis2

                def emit_at(p, scat_sb, dis2):
                    ds_ps = ds_pp.tile([60, 2, R, 60], f32, tag="ds")
                    for r in range(R):
                        for j in range(2):
                            row = dis2[0:1, j, 60 * r:60 * (r + 1)]
                            nc.tensor.matmul(
                                ds_ps[0:60, j, r, :],
                                lhsT=row,
                                rhs=row,
                                start=True, stop=True,
                            )
                    # j=1 block staged at partition base 64 so every SBUF
                    # tensor_tensor sees equal input base partitions
                    ds_sb = ds_sp.tile([128, R, 60], bf16, tag="ds_sb")
                    nc.scalar.copy(ds_sb[0:60, :, :], ds_ps[0:60, 0, :, :])
                    nc.scalar.copy(ds_sb[64:124, :, :], ds_ps[0:60, 1, :, :])

                    # AT = (scat + I) * (dis x dis), block-diagonal pair tiles
                    nc.gpsimd.tensor_tensor(
                        out=AT_all[0:60, p, :, 0:60],
                        in0=scat_sb[0:60, :, :],
                        in1=ds_sb[0:60, :, :],
                        op=OP.mult,
                    )
                    at1 = at1_pool.tile([128, R, 60], bf16, tag="at1")
                    nc.gpsimd.tensor_tensor(
                        out=at1[64:124, :, :],
                        in0=scat_sb[64:124, :, :],
                        in1=ds_sb[64:124, :, :],
                        op=OP.mult,
                    )
                    nc.gpsimd.dma_start(
                        out=AT_all[60:120, p, :, 60:120], in_=at1[64:124, :, :]
                    )

                # 1-pair software pipeline: PE never waits on the dis chain
                pending = None
                for p in range(npair):
                    h = emit_oh_scat(p)
                    if pending is not None:
                        emit_at(p - 1, *pending)
                    pending = h
                emit_at(npair - 1, *pending)

            # wf1 resident in bf16 (60KB/partition), streamed through a small
            # f32 staging pool with gpsimd converts during the layer phase
            wf1P = top.enter_context(tc.tile_pool(name="wf1", bufs=1))
            wf1bf = wf1P.tile([128, NKC, D], bf16)
            wstg = top.enter_context(tc.tile_pool(name="wstg", bufs=3))

            # =======================================================
            # Stage 3: GCN layers
            # =======================================================
            with ExitStack() as lp:
                y_pp = lp.enter_context(
                    tc.tile_pool(name="y_ps", bufs=2, space="PSUM")
                )
                y_sp = lp.enter_context(tc.tile_pool(name="y_sb", bufs=4))
                ms_pp = lp.enter_context(
                    tc.tile_pool(name="ms_ps", bufs=1, space="PSUM")
                )
                hm_sp = lp.enter_context(tc.tile_pool(name="hmid", bufs=2))
                h_pp = lp.enter_context(
                    tc.tile_pool(name="h_ps", bufs=2, space="PSUM")
                )

                nconv = 0
                for l in range(L):
                    nk = 1 if l == 0 else 2
                    hT[l + 1] = hT_pool.tile(
                        [128, 2, nn], f32r, tag="hT", name=f"hT{l + 1}"
                    )
                    pdone = 0
                    while pdone < npair:
                        # one wf1 group per layer-group iteration (12 total):
                        # DMA on the idle sync queue, convert on idle gpsimd
                        if nconv < NKC // 10:
                            i = nconv
                            stg = wstg.tile([128, 10, D], f32, tag="wstg")
                            nc.sync.dma_start(
                                out=stg[:],
                                in_=wf1_d[
                                    1280 * i:1280 * (i + 1), :
                                ].rearrange("(c p) d -> p c d", p=128),
                            )
                            if i % 2 == 0:
                                nc.vector.tensor_copy(
                                    out=wf1bf[:, 10 * i:10 * (i + 1), :],
                                    in_=stg[:],
                                )
                            else:
                                nc.scalar.copy(
                                    wf1bf[:, 10 * i:10 * (i + 1), :], stg[:]
                                )
                            nconv += 1
                        gs = min(4, npair - pdone)  # pairs in this group
                        ms = [
                            ms_pp.tile([128, 120 * gs], f32, tag=f"ms{mt}",
                                       name=f"ms{mt}")
                            for mt in range(2)
                        ]
                        for pp in range(gs):
                            p = pdone + pp
                            y_ps = y_pp.tile([128, R * D], f32, tag="y")
                            for fs in range(2):
                                for kc in range(nk):
                                    if l == 0:
                                        rhs = fs0_sb[:].rearrange("p r d -> p (r d)")
                                    else:
                                        rhs = fsg_sb[:, l - 1, kc].rearrange(
                                            "p r d -> p (r d)"
                                        )
                                    nc.tensor.matmul(
                                        y_ps[0:120, 512 * fs:512 * (fs + 1)],
                                        lhsT=hT[l][
                                            :, kc, 120 * p:120 * (p + 1)
                                        ],
                                        rhs=rhs[:, 512 * fs:512 * (fs + 1)],
                                        start=(kc == 0), stop=(kc == nk - 1),
                                    )
                            y_sb = y_sp.tile([128, R, D], bf16, tag="ysb")
                            if pp % 2 == 0:
                                nc.vector.tensor_copy(
                                    y_sb[0:120].rearrange("p r d -> p (r d)"),
                                    y_ps[0:120, :],
                                )
                            else:
                                nc.scalar.copy(
                                    y_sb[0:120].rearrange("p r d -> p (r d)"),
                                    y_ps[0:120, :],
                                )
                            for mt in range(2):
                                for r in range(R):
                                    nc.tensor.matmul(
                                        ms[mt][:, 120 * pp:120 * (pp + 1)],
                                        lhsT=y_sb[0:120, r, 128 * mt:128 * (mt + 1)],
                                        rhs=AT_all[:, p, r, :],
                                        start=(r == 0), stop=(r == R - 1),
                                    )
                        hmid = hm_sp.tile([128, 2, 120 * gs], f32r, tag="hmid")
                        for mt in range(2):
                            nc.scalar.activation(
                                hmid[:, mt, :], ms[mt][:], AF.Relu,
                                bias=c_sb[:, mt, l:l + 1],
                            )
                        for mt2 in range(2):
                            hp = h_pp.tile([128, 120 * gs], f32, tag="hp")
                            for kc in range(2):
                                nc.tensor.matmul(
                                    hp[:],
                                    lhsT=wi2r[
                                        :, kc, 128 * mt2:128 * (mt2 + 1)
                                    ],
                                    rhs=hmid[:, kc, :],
                                    start=(kc == 0), stop=(kc == 1),
                                )
                            nc.scalar.activation(
                                hT[l + 1][:, mt2, 120 * pdone:120 * (pdone + gs)],
                                hp[:], AF.Identity, bias=bi2_fm[:, mt2:mt2 + 1],
                            )
                        pdone += gs

            # =======================================================
            # Stage 4: final FC (z1^T accumulation over native wf1 chunks)
            # =======================================================
            with ExitStack() as fp_:
                z_pp = fp_.enter_context(
                    tc.tile_pool(name="z_ps", bufs=1, space="PSUM")
                )
                z_sp = fp_.enter_context(tc.tile_pool(name="z_sb", bufs=1))

                # bf16 copy of h3 for the bf16 z1 matmuls
                h3b = z_sp.tile([128, 2, nn], bf16)
                nc.vector.tensor_copy(h3b[:], hT[L][:])
                z1_ps = z_pp.tile([32, D], f32, tag="z1", name="z1")
                for ch in range(NKC):
                    dc, n = ch & 1, ch >> 1
                    lhsT = h3b[:, dc, :].rearrange("p (g n) -> p n g", n=N)[:, n, :]
                    nc.tensor.matmul(
                        z1_ps[0:gc, :],
                        lhsT=lhsT,
                        rhs=wf1bf[:, ch, :],
                        start=(ch == 0), stop=False,
                    )
                # bias row: z1 += 1 (x) bf1
                nc.tensor.matmul(
                    z1_ps[0:gc, :],
                    lhsT=ones1[0:1, :],
                    rhs=bf1_row[0:1, :],
                    start=False, stop=True,
                )
                z1_sb = z_sp.tile([32, D], f32)
                nc.scalar.activation(z1_sb[0:gc, :], z1_ps[0:gc, :], AF.Relu)

                # transpose z1 -> feature-major [128, 2, gc]
                z1T_sb = z_sp.tile([128, 2, gc], bf16)
                for m in range(2):
                    ztp = z_pp.tile([128, 32], f32, tag="ztp", name=f"ztp{m}")
                    nc.tensor.transpose(
                        ztp[:, 0:gc], z1_sb[0:gc, 128 * m:128 * (m + 1)],
                        ident[0:gc, 0:gc],
                    )
                    nc.scalar.copy(z1T_sb[:, m, :], ztp[:, 0:gc])

                z2_ps = [
                    z_pp.tile([128, gc], f32, tag=f"z2_{mt}", name=f"z2_{mt}")
                    for mt in range(2)
                ]
                for mt in range(2):
                    for kc in range(2):
                        nc.tensor.matmul(
                            z2_ps[mt][:],
                            lhsT=wf2b[:, kc, 128 * mt:128 * (mt + 1)],
                            rhs=z1T_sb[:, kc, :],
                            start=(kc == 0), stop=(kc == 1),
                        )
                z2_sb = z_sp.tile([128, 2, gc], bf16)
                for mt in range(2):
                    nc.scalar.activation(
                        z2_sb[:, mt, :], z2_ps[mt][:], AF.Relu,
                        bias=bf2_fm[:, mt:mt + 1],
                    )
                z3_ps = z_pp.tile([2, gc], f32, tag="z3")
                for kc in range(2):
                    nc.tensor.matmul(
                        z3_ps[0:2, :],
                        lhsT=wf3b[:, kc, :],
                        rhs=z2_sb[:, kc, :],
                        start=(kc == 0), stop=(kc == 1),
                    )
                out_sb = z_sp.tile([2, gc], f32)
                nc.scalar.activation(
                    out_sb[0:2, :], z3_ps[0:2, :], AF.Identity,
                    bias=bf3_fm[0:2, 0:1],
                )
                nc.sync.dma_start(
                    out=out_d.rearrange("g j -> j g"), in_=out_sb[0:2, :]
                )

    nc.compile()
    return nc


def shard_inputs(inputs, gc=GC, ncores=NCORES):
    """Full inputs -> per-core in_maps (host-side layout only)."""
    x = np.ascontiguousarray(inputs["x"], dtype=np.float32)
    ei = np.ascontiguousarray(inputs["edge_index"], dtype=np.int32)
    ew = np.ascontiguousarray(inputs["edge_weight"], dtype=np.float32)
    shared = {
        "w0": np.ascontiguousarray(inputs["W_gcn0"], np.float32),
        "wg": np.ascontiguousarray(inputs["W_gcn"], np.float32),
        "b0": np.ascontiguousarray(inputs["b_gcn0"], np.float32).reshape(-1),
        "bg": np.ascontiguousarray(inputs["b_gcn"], np.float32).reshape(L - 1, -1),
        "wi1": np.ascontiguousarray(inputs["Wi1"], np.float32),
        "bi1": np.ascontiguousarray(inputs["bi1"], np.float32),
        "wi2": np.ascontiguousarray(inputs["Wi2"], np.float32),
        "bi2": np.ascontiguousarray(inputs["bi2"], np.float32),
        "wf1": np.ascontiguousarray(inputs["Wf1"], np.float32),
        "bf1": np.ascontiguousarray(inputs["bf1"], np.float32),
        "wf2": np.ascontiguousarray(inputs["Wf2"], np.float32),
        "bf2": np.ascontiguousarray(inputs["bf2"], np.float32),
        "wf3": np.ascontiguousarray(inputs["Wf3"], np.float32),
        "bf3": np.ascontiguousarray(inputs["bf3"], np.float32),
    }
    in_maps = []
    for c in range(ncores):
        s = slice(c * gc, (c + 1) * gc)
        m = dict(shared)
        m["x"] = np.ascontiguousarray(x[s].reshape(gc * N, F))
        m["ei"] = np.ascontiguousarray(ei[s].reshape(gc * R * 2, E))
        m["ew"] = np.ascontiguousarray(ew[s].reshape(gc * R, E))
        in_maps.append(m)
    return in_maps


def kernel(**inputs):
    from concourse import bass_utils

    if "nc" not in _CACHE:
        _CACHE["nc"] = _build(GC)
    nc = _CACHE["nc"]
    in_maps = shard_inputs(inputs)
    res = bass_utils.run_bass_kernel_spmd(
        nc, in_maps, core_ids=list(range(NCORES))
    )
    return np.concatenate([r["out"] for r in res.results], axis=0)


# revision 41
# speedup vs baseline: 1.1096x; 1.1096x over previous
# kernel.py — Bass/Trainium2 kernel for nn_GCNBaseNet (gnn_message_passing)
#
# Sharding: data-parallel over graphs (8 cores x 32 graphs, replicated weights).
#
# Math restructuring (per layer, per graph):
#   reference:  h' = relu(concat_r(A_r h W_r + b_r) @ Wi1 + bi1) @ Wi2 + bi2
#   using concat_r(m_r) @ Wi1 = sum_r m_r @ Wi1_r  and A_r(h W_r) Wi1_r =
#   A_r (h (W_r Wi1_r)):
#       h' = relu(sum_r A_r (h @ Wfused_{l,r}) + c_l) @ Wi2 + bi2
#   with Wfused_{l,r} = W_{l,r} @ Wi1_r (computed on device) and
#   c_l = bi1 + sum_r b_{l,r} @ Wi1_r.
#
# Layout: activations are feature-major (hT: [D, nodes]) the whole way.
#
# A^T build (per pair of graphs, PE one-hot scatter):
#   - edge data loaded in NATIVE layout (2 contiguous DMAs), transposed
#     on-device via PE (int values exact in f32), converted to bf16.
#   - indices/weights duplicated into adjacent pairs (gpsimd copies) so the
#     one-hot is_equal and the x-w multiply run on DVE with every operand
#     2-byte stride-1 (DVE 2x fast mode) despite the broadcast.
#   - scatter via PE matmuls (one-hot^T @ one-hot), two graphs packed in PE
#     column groups; an extra all-w lhsT column yields the degree row free.
#   - per-pair: degree row -> ACT Rsqrt -> dis outer products (K=1 PE
#     matmuls) -> gpsimd elementwise (scat+I) * (dis x dis). No global
#     barrier; pairs pipeline across DVE/PE/ACT/gpsimd.
#
# Final FC: z1^T = sum_k hT3-slice^T @ Wf1-chunk with Wf1 in its native
# layout (1KB packets), bias added via a K=1 ones-row matmul, Wf1 prefetched
# in two big batches that hide under the A-build and layer phases.
import numpy as np

G, N, F, D, R, E, L = 256, 60, 128, 256, 4, 512, 3
NCORES = 8
GC = G // NCORES  # graphs per core
C = E // 128      # edge chunks per (g, r)

_CACHE = {}


def _build(gc, enable_asserts=False):
    """Builds the full Bass module for `gc` graphs on one core."""
    from contextlib import ExitStack

    import concourse.mybir as mybir
    import concourse.tile as tile
    from concourse import bacc
    from concourse.masks import make_identity

    dt = mybir.dt
    f32, f32r, bf16, i32 = dt.float32, dt.float32r, dt.bfloat16, dt.int32
    AF = mybir.ActivationFunctionType
    OP = mybir.AluOpType

    npair = gc // 2
    nn = gc * N                      # nodes per core
    nt = (nn + 127) // 128           # x row tiles
    NKC = (N * D) // 128             # wf1 K chunks (120)

    nc = bacc.Bacc(
        "TRN2",
        target_bir_lowering=False,
        debug=False,
        enable_asserts=enable_asserts,
        num_devices=NCORES,
    )

    # ---- DRAM tensors -----------------------------------------------------
    x_d = nc.dram_tensor("x", [nn, F], f32, kind="ExternalInput").ap()
    ei_d = nc.dram_tensor("ei", [gc * R * 2, E], i32, kind="ExternalInput").ap()
    ew_d = nc.dram_tensor("ew", [gc * R, E], f32, kind="ExternalInput").ap()
    w0_d = nc.dram_tensor("w0", [R, F, D], f32, kind="ExternalInput").ap()
    wg_d = nc.dram_tensor("wg", [L - 1, R, D, D], f32, kind="ExternalInput").ap()
    b0_d = nc.dram_tensor("b0", [R * D], f32, kind="ExternalInput").ap()
    bg_d = nc.dram_tensor("bg", [L - 1, R * D], f32, kind="ExternalInput").ap()
    wi1_d = nc.dram_tensor("wi1", [R * D, D], f32, kind="ExternalInput").ap()
    bi1_d = nc.dram_tensor("bi1", [D], f32, kind="ExternalInput").ap()
    wi2_d = nc.dram_tensor("wi2", [D, D], f32, kind="ExternalInput").ap()
    bi2_d = nc.dram_tensor("bi2", [D], f32, kind="ExternalInput").ap()
    wf1_d = nc.dram_tensor("wf1", [N * D, D], f32, kind="ExternalInput").ap()
    bf1_d = nc.dram_tensor("bf1", [D], f32, kind="ExternalInput").ap()
    wf2_d = nc.dram_tensor("wf2", [D, D], f32, kind="ExternalInput").ap()
    bf2_d = nc.dram_tensor("bf2", [D], f32, kind="ExternalInput").ap()
    wf3_d = nc.dram_tensor("wf3", [D, 2], f32, kind="ExternalInput").ap()
    bf3_d = nc.dram_tensor("bf3", [2], f32, kind="ExternalInput").ap()
    out_d = nc.dram_tensor("out", [gc, 2], f32, kind="ExternalOutput").ap()

    with tile.TileContext(nc) as tc:
        with ExitStack() as top:
            persist = top.enter_context(tc.tile_pool(name="persist", bufs=1))

            # ---- constants ----
            ident = persist.tile([128, 128], f32)
            make_identity(nc, ident[:])
            iota62 = persist.tile([128, 62], bf16)
            i60 = persist.tile([60, 60], bf16)
            nc.gpsimd.memset(i60[:], 0.0)
            nc.gpsimd.affine_select(
                out=i60[:], in_=i60[:], compare_op=OP.not_equal, fill=1.0,
                base=0, pattern=[[-1, 60]], channel_multiplier=1,
            )
            selfT = persist.tile([60, 61], bf16)
            nc.gpsimd.memset(selfT[:], 0.0)
            nc.gpsimd.affine_select(
                out=selfT[:, 0:60], in_=selfT[:, 0:60], compare_op=OP.not_equal,
                fill=1.0, base=0, pattern=[[-1, 60]], channel_multiplier=1,
            )
            nc.gpsimd.memset(selfT[:, 60:61], 1.0)
            ones1 = persist.tile([1, gc], f32)
            nc.gpsimd.memset(ones1[:], 1.0)


            # feature-major bias vectors [128, 2] (chunk-major)
            def load_fm(name, ap):
                t = persist.tile([128, 2], f32, name=name, tag=name)
                nc.sync.dma_start(out=t[:], in_=ap.rearrange("(m p) -> p m", p=128))
                return t

            bi2_fm = load_fm("bi2_fm", bi2_d)
            bf2_fm = load_fm("bf2_fm", bf2_d)
            bf1_row = persist.tile([1, D], f32)
            nc.sync.dma_start(out=bf1_row[:], in_=bf1_d[None, :])
            bf3_fm = persist.tile([2, 1], f32)
            nc.sync.dma_start(out=bf3_fm[:], in_=bf3_d[:, None])
            bi1_fm = load_fm("bi1_fm", bi1_d)

            # persistent weights (staging buffers live in the prep pool)
            wi2r = persist.tile([128, 2, D], f32r)
            wf2b = persist.tile([128, 2, D], bf16)
            wf3b = persist.tile([128, 2, 2], bf16)
            fs0_sb = persist.tile([128, R, D], f32r)           # Wfused layer 0
            fsg_sb = persist.tile([128, L - 1, 2, R, D], f32r)  # [l, kc, r, d]
            c_sb = persist.tile([128, 2, L], f32)              # fused bias

            # AT_all: normalized block-diagonal A^T per pair [src, pair, r, tgt]
            AT_all = persist.tile([120, npair, R, 120], bf16)
            nc.gpsimd.memset(AT_all[:], 0.0)

            # ---- hT pool ----
            hT_pool = top.enter_context(tc.tile_pool(name="hT", bufs=2))
            hT = [None] * (L + 1)

            # =======================================================
            # Stage 1: input prep (weights, x transpose, edge prep)
            # =======================================================
            with ExitStack() as prep:
                wld = prep.enter_context(tc.tile_pool(name="wld", bufs=1))
                wps = prep.enter_context(
                    tc.tile_pool(name="wps", bufs=2, space="PSUM")
                )

                # constants / small weights staged here, cast into persist
                iota_i = wld.tile([128, 62], i32)
                nc.gpsimd.iota(
                    iota_i[:], pattern=[[1, 62]], base=0, channel_multiplier=0
                )
                nc.vector.tensor_copy(iota62[:], iota_i[:])
                wi2_sb = wld.tile([128, 2, D], f32)
                nc.sync.dma_start(
                    out=wi2_sb[:], in_=wi2_d.rearrange("(c p) d -> p c d", p=128)
                )
                nc.vector.tensor_copy(wi2r[:], wi2_sb[:])
                wf2_sb = wld.tile([128, 2, D], f32)
                nc.sync.dma_start(
                    out=wf2_sb[:], in_=wf2_d.rearrange("(c p) d -> p c d", p=128)
                )
                nc.vector.tensor_copy(wf2b[:], wf2_sb[:])
                wf3_sb = wld.tile([128, 2, 2], f32)
                nc.sync.dma_start(
                    out=wf3_sb[:], in_=wf3_d.rearrange("(c p) j -> p c j", p=128)
                )
                nc.vector.tensor_copy(wf3b[:], wf3_sb[:])

                # -- edge data: native-layout DMAs --
                eiN = wld.tile([128, 2, E], i32)
                nc.sync.dma_start(
                    out=eiN[:], in_=ei_d.rearrange("(t p) e -> p t e", p=128)
                )
                ewN = wld.tile([128, E], f32)
                nc.sync.dma_start(out=ewN[:], in_=ew_d[:, :])
                # i32 -> f32 value-convert in place (bit widths match)
                eiNf = eiN[:].bitcast(f32)
                nc.vector.tensor_copy(eiNf, eiN[:])

                # PE transposes: eiTf[p, c, t, row], ewTf[p, c, row]
                eiTf = wld.tile([128, C, 2, 128], f32)
                ewTf = wld.tile([128, C, 128], f32)
                for t in range(2):
                    for c in range(C):
                        tp = wps.tile([128, 128], f32, tag="tp")
                        nc.tensor.transpose(
                            tp[:], eiNf[:, t, 128 * c:128 * (c + 1)], ident[:]
                        )
                        nc.scalar.copy(eiTf[:, c, t, :], tp[:])
                del eiNf
                for c in range(C):
                    tp = wps.tile([128, 128], f32, tag="tp")
                    nc.tensor.transpose(
                        tp[:], ewN[:, 128 * c:128 * (c + 1)], ident[:]
                    )
                    nc.scalar.copy(ewTf[:, c, :], tp[:])

                # duplicated-pair index/weight tables (enable DVE 2x mode):
                # eidx2[p, g, r, two, c, dup], ew2[p, g, r, c, dup]
                eidx2 = persist.tile([128, gc, R, 2, C, 2], bf16)
                ew2 = persist.tile([128, gc, R, C, 2], bf16)
                for t in range(2):
                    src = eiTf[:, :, t, :].rearrange(
                        "p c (g r two) -> p g r two c", g=16, r=R, two=2
                    )
                    for dup in range(2):
                        nc.vector.tensor_copy(
                            out=eidx2[:, 16 * t:16 * (t + 1), :, :, :, dup],
                            in_=src,
                        )
                for dup in range(2):
                    nc.vector.tensor_copy(
                        out=ew2[:, :, :, :, dup],
                        in_=ewTf[:].rearrange(
                            "p c (g r) -> p g r c", g=gc, r=R
                        ),
                    )

                # -- x load (one DMA) + transpose --> hT[0] --
                hT[0] = hT_pool.tile([128, 2, nn], f32r, tag="hT", name="hT0")
                xt = wld.tile([128, nt, 128], f32)
                nc.sync.dma_start(
                    out=xt[:],
                    in_=x_d.rearrange("(t p) f -> p t f", p=128),
                )
                for t in range(nt):
                    tp = wps.tile([128, 128], f32, tag="tp")
                    nc.tensor.transpose(tp[:], xt[:, t, :], ident[:])
                    nc.scalar.copy(hT[0][:, 0, 128 * t:128 * (t + 1)], tp[:])

                # -- weight prep --
                wi1_sb = wld.tile([128, 2 * R, D], f32)
                nc.scalar.dma_start(
                    out=wi1_sb[:], in_=wi1_d.rearrange("(c p) d -> p c d", p=128)
                )
                w0_sb = wld.tile([128, R, D], f32)
                nc.sync.dma_start(
                    out=w0_sb[:], in_=w0_d.rearrange("r p d -> p r d")
                )
                wg_sb = wld.tile([128, L - 1, R, 2, D], f32)
                nc.scalar.dma_start(
                    out=wg_sb[:],
                    in_=wg_d.rearrange("l r (c p) d -> p l r c d", p=128),
                )
                wi1r = wld.tile([128, 2 * R, D], f32r)
                nc.vector.tensor_copy(wi1r[:], wi1_sb[:])
                w0T_sb = wld.tile([128, R, 2, 128], f32r)
                wgT_sb = wld.tile([128, L - 1, R, 2, 2, 128], f32r)

                for r in range(R):
                    for j in range(2):
                        tp = wps.tile([128, 128], f32, tag="tp")
                        nc.tensor.transpose(
                            tp[:], w0_sb[:, r, 128 * j:128 * (j + 1)], ident[:]
                        )
                        nc.scalar.copy(w0T_sb[:, r, j, :], tp[:])
                for l in range(L - 1):
                    for r in range(R):
                        for ja in range(2):
                            for fb in range(2):
                                tp = wps.tile([128, 128], f32, tag="tp")
                                nc.tensor.transpose(
                                    tp[:],
                                    wg_sb[:, l, r, fb, 128 * ja:128 * (ja + 1)],
                                    ident[:],
                                )
                                nc.scalar.copy(wgT_sb[:, l, r, ja, fb, :], tp[:])

                # Wfused = (W^T).T @ Wi1_r  (K = inner D, accumulated)
                for r in range(R):
                    fpp = wps.tile([128, D], f32, tag="fp")
                    for jc in range(2):
                        nc.tensor.matmul(
                            fpp[:],
                            lhsT=w0T_sb[:, r, jc, :],
                            rhs=wi1r[:, 2 * r + jc, :],
                            start=(jc == 0), stop=(jc == 1),
                        )
                    nc.scalar.copy(fs0_sb[:, r, :], fpp[:])
                for l in range(L - 1):
                    for r in range(R):
                        for fb in range(2):
                            fpp = wps.tile([128, D], f32, tag="fp")
                            for jc in range(2):
                                nc.tensor.matmul(
                                    fpp[:],
                                    lhsT=wgT_sb[:, l, r, jc, fb, :],
                                    rhs=wi1r[:, 2 * r + jc, :],
                                    start=(jc == 0), stop=(jc == 1),
                                )
                            nc.scalar.copy(fsg_sb[:, l, fb, r, :], fpp[:])

                # c_l = bi1 + sum_r b_lr @ Wi1_r   (feature-major [128,1] x2)
                b_sb = wld.tile([128, 2 * R, 4], f32)
                nc.gpsimd.memset(b_sb[:], 0.0)
                nc.sync.dma_start(
                    out=b_sb[:, :, 0:1],
                    in_=b0_d.rearrange("(c p) -> p c", p=128)[:, :, None],
                )
                for l in range(L - 1):
                    nc.sync.dma_start(
                        out=b_sb[:, :, l + 1:l + 2],
                        in_=bg_d[l].rearrange("(c p) -> p c", p=128)[:, :, None],
                    )
                b_sbr = wld.tile([128, 2 * R, 4], f32r)
                nc.vector.tensor_copy(b_sbr[:], b_sb[:])
                for m in range(2):
                    cp = wps.tile([128, 4], f32, tag="cp")
                    for ch in range(2 * R):
                        nc.tensor.matmul(
                            cp[:],
                            lhsT=wi1r[:, ch, 128 * m:128 * (m + 1)],
                            rhs=b_sbr[:, ch, :],
                            start=(ch == 0), stop=(ch == 2 * R - 1),
                        )
                    nc.scalar.activation(
                        c_sb[:, m, :], cp[:, 0:L], AF.Identity,
                        bias=bi1_fm[:, m:m + 1],
                    )

            # =======================================================
            # Stage 2: A-build (per-pair pipeline, no global barrier)
            # =======================================================
            with ExitStack() as ab:
                oh_pool = ab.enter_context(tc.tile_pool(name="oh", bufs=4))
                scat_sp = ab.enter_context(tc.tile_pool(name="scat_sb", bufs=1))
                stg_pool = ab.enter_context(tc.tile_pool(name="stg", bufs=1))
                ds_sp = ab.enter_context(tc.tile_pool(name="ds_sb", bufs=3))
                at1_pool = ab.enter_context(tc.tile_pool(name="at1", bufs=3))
                scat_pp = ab.enter_context(
                    tc.tile_pool(name="scat_ps", bufs=2, space="PSUM")
                )
                ds_pp = ab.enter_context(
                    tc.tile_pool(name="ds_ps", bufs=2, space="PSUM")
                )

                # all pairs' scatter results + degree rows stay resident so
                # the dis computation batches into ONE Ln + ONE Exp (pairs on
                # partitions) — 2 ACT table loads total instead of 32
                scat_all = scat_sp.tile([128, npair, R, 60], bf16)
                deg_all = stg_pool.tile([npair, 2, R * 60], bf16)
                dis_all = stg_pool.tile([npair, 2, R * 60], f32)

                def emit_oh_scat(p):
                    ohs = []
                    for j in range(2):
                        g = 2 * p + j
                        oh = oh_pool.tile(
                            [128, R, 2, C, 62], bf16, tag="oh", name=f"oh{j}"
                        )
                        # one-hot compare, all operands 2-byte stride-1
                        nc.vector.tensor_tensor(
                            out=oh[:].rearrange(
                                "p r two c (s2 d2) -> p (r two c) s2 d2", s2=31
                            ),
                            in0=iota62[:].rearrange(
                                "p (s2 d2) -> p s2 d2", s2=31
                            )[:, None, :, :].to_broadcast([128, R * 2 * C, 31, 2]),
                            in1=eidx2[:, g, :, :, :, :].rearrange(
                                "p r two c d2 -> p (r two c) d2"
                            )[:, :, None, :].to_broadcast([128, R * 2 * C, 31, 2]),
                            op=OP.is_equal,
                        )
                        # x w on the src half (cols 0..59)
                        nc.vector.tensor_tensor(
                            out=oh[:, :, 0, :, 0:60].rearrange(
                                "p r c (s2 d2) -> p r c s2 d2", s2=30
                            ),
                            in0=oh[:, :, 0, :, 0:60].rearrange(
                                "p r c (s2 d2) -> p r c s2 d2", s2=30
                            ),
                            in1=ew2[:, g, :, :, :][:, :, :, None, :].to_broadcast(
                                [128, R, C, 30, 2]
                            ),
                            op=OP.mult,
                        )
                        # w column (degree row source)
                        nc.vector.tensor_copy(
                            out=oh[:, :, 0, :, 60:61],
                            in_=ew2[:, g, :, :, 0:1],
                        )
                        ohs.append(oh)

                    scat_ps = scat_pp.tile([128, R, 60], f32, tag="scat")
                    for r in range(R):
                        for c in range(C):
                            for j in range(2):
                                nc.tensor.matmul(
                                    scat_ps[64 * j:64 * j + 61, r, :],
                                    lhsT=ohs[j][:, r, 0, c, 0:61],
                                    rhs=ohs[j][:, r, 1, c, 0:60],
                                    start=(c == 0), stop=False,
                                    tile_position=(0, 64 * j),
                                )
                        for j in range(2):
                            nc.tensor.matmul(
                                scat_ps[64 * j:64 * j + 61, r, :],
                                lhsT=selfT[:],
                                rhs=i60[:],
                                start=False, stop=True,
                                tile_position=(0, 64 * j),
                            )
                    nc.scalar.copy(scat_all[:, p, :, :], scat_ps[:])
                    for j in range(2):
                        nc.gpsimd.dma_start(
                            out=deg_all[p:p + 1, j, :],
                            in_=scat_all[64 * j + 60:64 * j + 61, p, :, :],
                        )

                def emit_at(p):
                    # K=1 matmul lhsT must sit at partition 0 — restage row p
                    dis_p = ds_sp.tile([1, 2, R * 60], f32, tag="dis_p")
                    nc.gpsimd.dma_start(out=dis_p[0:1], in_=dis_all[p:p + 1])
                    ds_ps = ds_pp.tile([60, 2, R, 60], f32, tag="ds")
                    for r in range(R):
                        for j in range(2):
                            row = dis_p[0:1, j, 60 * r:60 * (r + 1)]
                            nc.tensor.matmul(
                                ds_ps[0:60, j, r, :],
                                lhsT=row,
                                rhs=row,
                                start=True, stop=True,
                            )
                    # j=1 block staged at partition base 64 so every SBUF
                    # tensor_tensor sees equal input base partitions
                    ds_sb = ds_sp.tile([128, R, 60], bf16, tag="ds_sb")
                    nc.scalar.copy(ds_sb[0:60, :, :], ds_ps[0:60, 0, :, :])
                    nc.scalar.copy(ds_sb[64:124, :, :], ds_ps[0:60, 1, :, :])

                    # AT = (scat + I) * (dis x dis), block-diagonal pair tiles
                    nc.gpsimd.tensor_tensor(
                        out=AT_all[0:60, p, :, 0:60],
                        in0=scat_all[0:60, p, :, :],
                        in1=ds_sb[0:60, :, :],
                        op=OP.mult,
                    )
                    at1 = at1_pool.tile([128, R, 60], bf16, tag="at1")
                    nc.gpsimd.tensor_tensor(
                        out=at1[64:124, :, :],
                        in0=scat_all[64:124, p, :, :],
                        in1=ds_sb[64:124, :, :],
                        op=OP.mult,
                    )
                    nc.gpsimd.dma_start(
                        out=AT_all[60:120, p, :, 60:120], in_=at1[64:124, :, :]
                    )

                for p in range(npair):
                    emit_oh_scat(p)
                # one Ln + one Exp for all pairs: dis = exp(-0.5 ln deg)
                nc.scalar.activation(dis_all[:], deg_all[:], AF.Ln)
                nc.scalar.activation(
                    dis_all[:], dis_all[:], AF.Exp, scale=-0.5
                )
                for p in range(npair):
                    emit_at(p)

            # wf1 resident in bf16 (60KB/partition), streamed through a small
            # f32 staging pool with gpsimd converts during the layer phase
            wf1P = top.enter_context(tc.tile_pool(name="wf1", bufs=1))
            wf1bf = wf1P.tile([128, NKC, D], bf16)
            wstg = top.enter_context(tc.tile_pool(name="wstg", bufs=3))

            # =======================================================
            # Stage 3: GCN layers
            # =======================================================
            with ExitStack() as lp:
                y_pp = lp.enter_context(
                    tc.tile_pool(name="y_ps", bufs=2, space="PSUM")
                )
                y_sp = lp.enter_context(tc.tile_pool(name="y_sb", bufs=4))
                ms_pp = lp.enter_context(
                    tc.tile_pool(name="ms_ps", bufs=1, space="PSUM")
                )
                hm_sp = lp.enter_context(tc.tile_pool(name="hmid", bufs=2))
                h_pp = lp.enter_context(
                    tc.tile_pool(name="h_ps", bufs=2, space="PSUM")
                )

                nconv = 0
                for l in range(L):
                    nk = 1 if l == 0 else 2
                    hT[l + 1] = hT_pool.tile(
                        [128, 2, nn], f32r, tag="hT", name=f"hT{l + 1}"
                    )
                    pdone = 0
                    while pdone < npair:
                        # one wf1 group per layer-group iteration (12 total):
                        # DMA on the idle sync queue, convert on idle gpsimd
                        if nconv < NKC // 10:
                            i = nconv
                            stg = wstg.tile([128, 10, D], f32, tag="wstg")
                            nc.sync.dma_start(
                                out=stg[:],
                                in_=wf1_d[
                                    1280 * i:1280 * (i + 1), :
                                ].rearrange("(c p) d -> p c d", p=128),
                            )
                            if i % 2 == 0:
                                nc.vector.tensor_copy(
                                    out=wf1bf[:, 10 * i:10 * (i + 1), :],
                                    in_=stg[:],
                                )
                            else:
                                nc.scalar.copy(
                                    wf1bf[:, 10 * i:10 * (i + 1), :], stg[:]
                                )
                            nconv += 1
                        gs = min(4, npair - pdone)  # pairs in this group
                        ms = [
                            ms_pp.tile([128, 120 * gs], f32, tag=f"ms{mt}",
                                       name=f"ms{mt}")
                            for mt in range(2)
                        ]
                        for pp in range(gs):
                            p = pdone + pp
                            y_ps = y_pp.tile([128, R * D], f32, tag="y")
                            for fs in range(2):
                                for kc in range(nk):
                                    if l == 0:
                                        rhs = fs0_sb[:].rearrange("p r d -> p (r d)")
                                    else:
                                        rhs = fsg_sb[:, l - 1, kc].rearrange(
                                            "p r d -> p (r d)"
                                        )
                                    nc.tensor.matmul(
                                        y_ps[0:120, 512 * fs:512 * (fs + 1)],
                                        lhsT=hT[l][
                                            :, kc, 120 * p:120 * (p + 1)
                                        ],
                                        rhs=rhs[:, 512 * fs:512 * (fs + 1)],
                                        start=(kc == 0), stop=(kc == nk - 1),
                                    )
                            y_sb = y_sp.tile([128, R, D], bf16, tag="ysb")
                            if pp % 2 == 0:
                                nc.vector.tensor_copy(
                                    y_sb[0:120].rearrange("p r d -> p (r d)"),
                                    y_ps[0:120, :],
                                )
                            else:
                                nc.scalar.copy(
                                    y_sb[0:120].rearrange("p r d -> p (r d)"),
                                    y_ps[0:120, :],
                                )
                            for mt in range(2):
                                for r in range(R):
                                    nc.tensor.matmul(
                                        ms[mt][:, 120 * pp:120 * (pp + 1)],
                                        lhsT=y_sb[0:120, r, 128 * mt:128 * (mt + 1)],
                                        rhs=AT_all[:, p, r, :],
                                        start=(r == 0), stop=(r == R - 1),
                                    )
                        hmid = hm_sp.tile([128, 2, 120 * gs], f32r, tag="hmid")
                        for mt in range(2):
                            nc.scalar.activation(
                                hmid[:, mt, :], ms[mt][:], AF.Relu,
                                bias=c_sb[:, mt, l:l + 1],
                            )
                        for mt2 in range(2):
                            hp = h_pp.tile([128, 120 * gs], f32, tag="hp")
                            for kc in range(2):
                                nc.tensor.matmul(
                                    hp[:],
                                    lhsT=wi2r[
                                        :, kc, 128 * mt2:128 * (mt2 + 1)
                                    ],
                                    rhs=hmid[:, kc, :],
                                    start=(kc == 0), stop=(kc == 1),
                                )
                            nc.scalar.activation(
                                hT[l + 1][:, mt2, 120 * pdone:120 * (pdone + gs)],
                                hp[:], AF.Identity, bias=bi2_fm[:, mt2:mt2 + 1],
                            )
                        pdone += gs

            # =======================================================
            # Stage 4: final FC (z1^T accumulation over native wf1 chunks)
            # =======================================================
            with ExitStack() as fp_:
                z_pp = fp_.enter_context(
                    tc.tile_pool(name="z_ps", bufs=1, space="PSUM")
                )
                z_sp = fp_.enter_context(tc.tile_pool(name="z_sb", bufs=1))

                # bf16 copy of h3 for the bf16 z1 matmuls
                h3b = z_sp.tile([128, 2, nn], bf16)
                nc.vector.tensor_copy(h3b[:], hT[L][:])
                z1_ps = z_pp.tile([32, D], f32, tag="z1", name="z1")
                for ch in range(NKC):
                    dc, n = ch & 1, ch >> 1
                    lhsT = h3b[:, dc, :].rearrange("p (g n) -> p n g", n=N)[:, n, :]
                    nc.tensor.matmul(
                        z1_ps[0:gc, :],
                        lhsT=lhsT,
                        rhs=wf1bf[:, ch, :],
                        start=(ch == 0), stop=False,
                    )
                # bias row: z1 += 1 (x) bf1
                nc.tensor.matmul(
                    z1_ps[0:gc, :],
                    lhsT=ones1[0:1, :],
                    rhs=bf1_row[0:1, :],
                    start=False, stop=True,
                )
                z1_sb = z_sp.tile([32, D], f32)
                nc.scalar.activation(z1_sb[0:gc, :], z1_ps[0:gc, :], AF.Relu)

                # transpose z1 -> feature-major [128, 2, gc]
                z1T_sb = z_sp.tile([128, 2, gc], bf16)
                for m in range(2):
                    ztp = z_pp.tile([128, 32], f32, tag="ztp", name=f"ztp{m}")
                    nc.tensor.transpose(
                        ztp[:, 0:gc], z1_sb[0:gc, 128 * m:128 * (m + 1)],
                        ident[0:gc, 0:gc],
                    )
                    nc.scalar.copy(z1T_sb[:, m, :], ztp[:, 0:gc])

                z2_ps = [
                    z_pp.tile([128, gc], f32, tag=f"z2_{mt}", name=f"z2_{mt}")
                    for mt in range(2)
                ]
                for mt in range(2):
                    for kc in range(2):
                        nc.tensor.matmul(
                            z2_ps[mt][:],
                            lhsT=wf2b[:, kc, 128 * mt:128 * (mt + 1)],
                            rhs=z1T_sb[:, kc, :],
                            start=(kc == 0), stop=(kc == 1),
                        )
                z2_sb = z_sp.tile([128, 2, gc], bf16)
                for mt in range(2):
                    nc.scalar.activation(
                        z2_sb[:, mt, :], z2_ps[mt][:], AF.Relu,
                        bias=bf2_fm[:, mt:mt + 1],
                    )
                z3_ps = z_pp.tile([2, gc], f32, tag="z3")
                for kc in range(2):
                    nc.tensor.matmul(
                        z3_ps[0:2, :],
                        lhsT=wf3b[:, kc, :],
                        rhs=z2_sb[:, kc, :],
                        start=(kc == 0), stop=(kc == 1),
                    )
                out_sb = z_sp.tile([2, gc], f32)
                nc.scalar.activation(
                    out_sb[0:2, :], z3_ps[0:2, :], AF.Identity,
                    bias=bf3_fm[0:2, 0:1],
                )
                nc.sync.dma_start(
                    out=out_d.rearrange("g j -> j g"), in_=out_sb[0:2, :]
                )

    nc.compile()
    return nc


def shard_inputs(inputs, gc=GC, ncores=NCORES):
    """Full inputs -> per-core in_maps (host-side layout only)."""
    x = np.ascontiguousarray(inputs["x"], dtype=np.float32)
    ei = np.ascontiguousarray(inputs["edge_index"], dtype=np.int32)
    ew = np.ascontiguousarray(inputs["edge_weight"], dtype=np.float32)
    shared = {
        "w0": np.ascontiguousarray(inputs["W_gcn0"], np.float32),
        "wg": np.ascontiguousarray(inputs["W_gcn"], np.float32),
        "b0": np.ascontiguousarray(inputs["b_gcn0"], np.float32).reshape(-1),
        "bg": np.ascontiguousarray(inputs["b_gcn"], np.float32).reshape(L - 1, -1),
        "wi1": np.ascontiguousarray(inputs["Wi1"], np.float32),
        "bi1": np.ascontiguousarray(inputs["bi1"], np.float32),
        "wi2": np.ascontiguousarray(inputs["Wi2"], np.float32),
        "bi2": np.ascontiguousarray(inputs["bi2"], np.float32),
        "wf1": np.ascontiguousarray(inputs["Wf1"], np.float32),
        "bf1": np.ascontiguousarray(inputs["bf1"], np.float32),
        "wf2": np.ascontiguousarray(inputs["Wf2"], np.float32),
        "bf2": np.ascontiguousarray(inputs["bf2"], np.float32),
        "wf3": np.ascontiguousarray(inputs["Wf3"], np.float32),
        "bf3": np.ascontiguousarray(inputs["bf3"], np.float32),
    }
    in_maps = []
    for c in range(ncores):
        s = slice(c * gc, (c + 1) * gc)
        m = dict(shared)
        m["x"] = np.ascontiguousarray(x[s].reshape(gc * N, F))
        m["ei"] = np.ascontiguousarray(ei[s].reshape(gc * R * 2, E))
        m["ew"] = np.ascontiguousarray(ew[s].reshape(gc * R, E))
        in_maps.append(m)
    return in_maps


def kernel(**inputs):
    from concourse import bass_utils

    if "nc" not in _CACHE:
        _CACHE["nc"] = _build(GC)
    nc = _CACHE["nc"]
    in_maps = shard_inputs(inputs)
    res = bass_utils.run_bass_kernel_spmd(
        nc, in_maps, core_ids=list(range(NCORES))
    )
    return np.concatenate([r["out"] for r in res.results], axis=0)


# revision 47
# speedup vs baseline: 1.1494x; 1.0359x over previous
# kernel.py — Bass/Trainium2 kernel for nn_GCNBaseNet (gnn_message_passing)
#
# Sharding: data-parallel over graphs (8 cores x 32 graphs, replicated weights).
#
# Math restructuring (per layer, per graph):
#   reference:  h' = relu(concat_r(A_r h W_r + b_r) @ Wi1 + bi1) @ Wi2 + bi2
#   using concat_r(m_r) @ Wi1 = sum_r m_r @ Wi1_r  and A_r(h W_r) Wi1_r =
#   A_r (h (W_r Wi1_r)):
#       h' = relu(sum_r A_r (h @ Wfused_{l,r}) + c_l) @ Wi2 + bi2
#   with Wfused_{l,r} = W_{l,r} @ Wi1_r (computed on device) and
#   c_l = bi1 + sum_r b_{l,r} @ Wi1_r.
#
# Layout: activations are feature-major (hT: [D, nodes]) the whole way.
#
# A^T build (per pair of graphs, PE one-hot scatter):
#   - edge data loaded in NATIVE layout (2 contiguous DMAs), transposed
#     on-device via PE (int values exact in f32), converted to bf16.
#   - indices/weights duplicated into adjacent pairs (gpsimd copies) so the
#     one-hot is_equal and the x-w multiply run on DVE with every operand
#     2-byte stride-1 (DVE 2x fast mode) despite the broadcast.
#   - scatter via PE matmuls (one-hot^T @ one-hot), two graphs packed in PE
#     column groups; an extra all-w lhsT column yields the degree row free.
#   - per-pair: degree row -> ACT Rsqrt -> dis outer products (K=1 PE
#     matmuls) -> gpsimd elementwise (scat+I) * (dis x dis). No global
#     barrier; pairs pipeline across DVE/PE/ACT/gpsimd.
#
# Final FC: z1^T = sum_k hT3-slice^T @ Wf1-chunk with Wf1 in its native
# layout (1KB packets), bias added via a K=1 ones-row matmul, Wf1 prefetched
# in two big batches that hide under the A-build and layer phases.
import numpy as np

G, N, F, D, R, E, L = 256, 60, 128, 256, 4, 512, 3
NCORES = 8
GC = G // NCORES  # graphs per core
C = E // 128      # edge chunks per (g, r)

_CACHE = {}


def _build(gc, enable_asserts=False):
    """Builds the full Bass module for `gc` graphs on one core."""
    from contextlib import ExitStack

    import concourse.mybir as mybir
    import concourse.tile as tile
    from concourse import bacc
    from concourse.masks import make_identity

    dt = mybir.dt
    f32, f32r, bf16, i32 = dt.float32, dt.float32r, dt.bfloat16, dt.int32
    AF = mybir.ActivationFunctionType
    OP = mybir.AluOpType

    npair = gc // 2
    nn = gc * N                      # nodes per core
    nt = (nn + 127) // 128           # x row tiles
    NKC = (N * D) // 128             # wf1 K chunks (120)

    nc = bacc.Bacc(
        "TRN2",
        target_bir_lowering=False,
        debug=False,
        enable_asserts=enable_asserts,
        num_devices=NCORES,
    )

    # ---- DRAM tensors -----------------------------------------------------
    x_d = nc.dram_tensor("x", [nn, F], f32, kind="ExternalInput").ap()
    ei_d = nc.dram_tensor("ei", [gc * R * 2, E], i32, kind="ExternalInput").ap()
    ew_d = nc.dram_tensor("ew", [gc * R, E], f32, kind="ExternalInput").ap()
    w0_d = nc.dram_tensor("w0", [R, F, D], f32, kind="ExternalInput").ap()
    wg_d = nc.dram_tensor("wg", [L - 1, R, D, D], f32, kind="ExternalInput").ap()
    b0_d = nc.dram_tensor("b0", [R * D], f32, kind="ExternalInput").ap()
    bg_d = nc.dram_tensor("bg", [L - 1, R * D], f32, kind="ExternalInput").ap()
    wi1_d = nc.dram_tensor("wi1", [R * D, D], f32, kind="ExternalInput").ap()
    bi1_d = nc.dram_tensor("bi1", [D], f32, kind="ExternalInput").ap()
    wi2_d = nc.dram_tensor("wi2", [D, D], f32, kind="ExternalInput").ap()
    bi2_d = nc.dram_tensor("bi2", [D], f32, kind="ExternalInput").ap()
    wf1_d = nc.dram_tensor("wf1", [N * D, D], f32, kind="ExternalInput").ap()
    bf1_d = nc.dram_tensor("bf1", [D], f32, kind="ExternalInput").ap()
    wf2_d = nc.dram_tensor("wf2", [D, D], f32, kind="ExternalInput").ap()
    bf2_d = nc.dram_tensor("bf2", [D], f32, kind="ExternalInput").ap()
    wf3_d = nc.dram_tensor("wf3", [D, 2], f32, kind="ExternalInput").ap()
    bf3_d = nc.dram_tensor("bf3", [2], f32, kind="ExternalInput").ap()
    out_d = nc.dram_tensor("out", [gc, 2], f32, kind="ExternalOutput").ap()

    with tile.TileContext(nc) as tc:
        with ExitStack() as top:
            persist = top.enter_context(tc.tile_pool(name="persist", bufs=1))

            # ---- constants ----
            ident = persist.tile([128, 128], f32)
            make_identity(nc, ident[:])
            iota62 = persist.tile([128, 62], bf16)
            i60 = persist.tile([60, 60], bf16)
            nc.gpsimd.memset(i60[:], 0.0)
            nc.gpsimd.affine_select(
                out=i60[:], in_=i60[:], compare_op=OP.not_equal, fill=1.0,
                base=0, pattern=[[-1, 60]], channel_multiplier=1,
            )
            selfT = persist.tile([60, 61], bf16)
            nc.gpsimd.memset(selfT[:], 0.0)
            nc.gpsimd.affine_select(
                out=selfT[:, 0:60], in_=selfT[:, 0:60], compare_op=OP.not_equal,
                fill=1.0, base=0, pattern=[[-1, 60]], channel_multiplier=1,
            )
            nc.gpsimd.memset(selfT[:, 60:61], 1.0)
            ones1 = persist.tile([1, gc], f32)
            nc.gpsimd.memset(ones1[:], 1.0)


            # feature-major bias vectors [128, 2] (chunk-major)
            def load_fm(name, ap):
                t = persist.tile([128, 2], f32, name=name, tag=name)
                nc.sync.dma_start(out=t[:], in_=ap.rearrange("(m p) -> p m", p=128))
                return t

            bi2_fm = load_fm("bi2_fm", bi2_d)
            bf2_fm = load_fm("bf2_fm", bf2_d)
            bf1_row = persist.tile([1, D], f32)
            nc.sync.dma_start(out=bf1_row[:], in_=bf1_d[None, :])
            bf3_fm = persist.tile([2, 1], f32)
            nc.sync.dma_start(out=bf3_fm[:], in_=bf3_d[:, None])
            bi1_fm = load_fm("bi1_fm", bi1_d)

            # persistent weights (staging buffers live in the prep pool)
            wi2r = persist.tile([128, 2, D], f32r)
            wf2b = persist.tile([128, 2, D], bf16)
            wf3b = persist.tile([128, 2, 2], bf16)
            fs0_sb = persist.tile([128, R, D], f32r)           # Wfused layer 0
            fsg_sb = persist.tile([128, L - 1, 2, R, D], f32r)  # [l, kc, r, d]
            c_sb = persist.tile([128, 2, L], f32)              # fused bias

            # AT_all: normalized block-diagonal A^T per pair [src, pair, r, tgt]
            AT_all = persist.tile([120, npair, R, 120], bf16)
            nc.gpsimd.memset(AT_all[:], 0.0)

            # ---- hT pool ----
            hT_pool = top.enter_context(tc.tile_pool(name="hT", bufs=2))
            hT = [None] * (L + 1)

            # =======================================================
            # Stage 1: input prep (weights, x transpose, edge prep)
            # =======================================================
            with ExitStack() as prep:
                wld = prep.enter_context(tc.tile_pool(name="wld", bufs=1))
                wps = prep.enter_context(
                    tc.tile_pool(name="wps", bufs=2, space="PSUM")
                )

                # -- edge data: native-layout DMAs (first — they gate the
                # A-build's DVE stream) --
                eiN = wld.tile([128, 2, E], i32)
                nc.sync.dma_start(
                    out=eiN[:], in_=ei_d.rearrange("(t p) e -> p t e", p=128)
                )
                ewN = wld.tile([128, E], f32)
                nc.sync.dma_start(out=ewN[:], in_=ew_d[:, :])
                # i32 -> f32 value-convert in place (bit widths match)
                eiNf = eiN[:].bitcast(f32)
                nc.vector.tensor_copy(eiNf, eiN[:])
                iota_i = wld.tile([128, 62], i32)
                nc.gpsimd.iota(
                    iota_i[:], pattern=[[1, 62]], base=0, channel_multiplier=0
                )
                nc.vector.tensor_copy(iota62[:], iota_i[:])

                # PE transposes: eiTf[p, c, t, row], ewTf[p, c, row]
                eiTf = wld.tile([128, C, 2, 128], f32)
                ewTf = wld.tile([128, C, 128], f32)
                for t in range(2):
                    for c in range(C):
                        tp = wps.tile([128, 128], f32, tag="tp")
                        nc.tensor.transpose(
                            tp[:], eiNf[:, t, 128 * c:128 * (c + 1)], ident[:]
                        )
                        nc.scalar.copy(eiTf[:, c, t, :], tp[:])
                del eiNf
                for c in range(C):
                    tp = wps.tile([128, 128], f32, tag="tp")
                    nc.tensor.transpose(
                        tp[:], ewN[:, 128 * c:128 * (c + 1)], ident[:]
                    )
                    nc.scalar.copy(ewTf[:, c, :], tp[:])

                # duplicated-pair index/weight tables (enable DVE 2x mode):
                # eidx2[p, g, r, two, c, dup], ew2[p, g, r, c, dup]
                eidx2 = persist.tile([128, gc, R, 2, C, 2], bf16)
                ew2 = persist.tile([128, gc, R, C, 2], bf16)
                for t in range(2):
                    src = eiTf[:, :, t, :].rearrange(
                        "p c (g r two) -> p g r two c", g=16, r=R, two=2
                    )
                    for dup in range(2):
                        nc.vector.tensor_copy(
                            out=eidx2[:, 16 * t:16 * (t + 1), :, :, :, dup],
                            in_=src,
                        )
                for dup in range(2):
                    nc.vector.tensor_copy(
                        out=ew2[:, :, :, :, dup],
                        in_=ewTf[:].rearrange(
                            "p c (g r) -> p g r c", g=gc, r=R
                        ),
                    )

                # -- x load (one DMA) + transpose --> hT[0] --
                hT[0] = hT_pool.tile([128, 2, nn], f32r, tag="hT", name="hT0")
                xt = wld.tile([128, nt, 128], f32)
                nc.sync.dma_start(
                    out=xt[:],
                    in_=x_d.rearrange("(t p) f -> p t f", p=128),
                )
                for t in range(nt):
                    tp = wps.tile([128, 128], f32, tag="tp")
                    nc.tensor.transpose(tp[:], xt[:, t, :], ident[:])
                    nc.scalar.copy(hT[0][:, 0, 128 * t:128 * (t + 1)], tp[:])

                # -- weight prep --
                wi2_sb = wld.tile([128, 2, D], f32)
                nc.sync.dma_start(
                    out=wi2_sb[:], in_=wi2_d.rearrange("(c p) d -> p c d", p=128)
                )
                nc.vector.tensor_copy(wi2r[:], wi2_sb[:])
                wf2_sb = wld.tile([128, 2, D], f32)
                nc.sync.dma_start(
                    out=wf2_sb[:], in_=wf2_d.rearrange("(c p) d -> p c d", p=128)
                )
                nc.vector.tensor_copy(wf2b[:], wf2_sb[:])
                wf3_sb = wld.tile([128, 2, 2], f32)
                nc.sync.dma_start(
                    out=wf3_sb[:], in_=wf3_d.rearrange("(c p) j -> p c j", p=128)
                )
                nc.vector.tensor_copy(wf3b[:], wf3_sb[:])
                wi1_sb = wld.tile([128, 2 * R, D], f32)
                nc.scalar.dma_start(
                    out=wi1_sb[:], in_=wi1_d.rearrange("(c p) d -> p c d", p=128)
                )
                w0_sb = wld.tile([128, R, D], f32)
                nc.sync.dma_start(
                    out=w0_sb[:], in_=w0_d.rearrange("r p d -> p r d")
                )
                wg_sb = wld.tile([128, L - 1, R, 2, D], f32)
                nc.scalar.dma_start(
                    out=wg_sb[:],
                    in_=wg_d.rearrange("l r (c p) d -> p l r c d", p=128),
                )
                wi1r = wld.tile([128, 2 * R, D], f32r)
                nc.vector.tensor_copy(wi1r[:], wi1_sb[:])
                w0T_sb = wld.tile([128, R, 2, 128], f32r)
                wgT_sb = wld.tile([128, L - 1, R, 2, 2, 128], f32r)

                for r in range(R):
                    for j in range(2):
                        tp = wps.tile([128, 128], f32, tag="tp")
                        nc.tensor.transpose(
                            tp[:], w0_sb[:, r, 128 * j:128 * (j + 1)], ident[:]
                        )
                        nc.scalar.copy(w0T_sb[:, r, j, :], tp[:])
                for l in range(L - 1):
                    for r in range(R):
                        for ja in range(2):
                            for fb in range(2):
                                tp = wps.tile([128, 128], f32, tag="tp")
                                nc.tensor.transpose(
                                    tp[:],
                                    wg_sb[:, l, r, fb, 128 * ja:128 * (ja + 1)],
                                    ident[:],
                                )
                                nc.scalar.copy(wgT_sb[:, l, r, ja, fb, :], tp[:])

                # Wfused = (W^T).T @ Wi1_r  (K = inner D, accumulated)
                for r in range(R):
                    fpp = wps.tile([128, D], f32, tag="fp")
                    for jc in range(2):
                        nc.tensor.matmul(
                            fpp[:],
                            lhsT=w0T_sb[:, r, jc, :],
                            rhs=wi1r[:, 2 * r + jc, :],
                            start=(jc == 0), stop=(jc == 1),
                        )
                    nc.scalar.copy(fs0_sb[:, r, :], fpp[:])
                for l in range(L - 1):
                    for r in range(R):
                        for fb in range(2):
                            fpp = wps.tile([128, D], f32, tag="fp")
                            for jc in range(2):
                                nc.tensor.matmul(
                                    fpp[:],
                                    lhsT=wgT_sb[:, l, r, jc, fb, :],
                                    rhs=wi1r[:, 2 * r + jc, :],
                                    start=(jc == 0), stop=(jc == 1),
                                )
                            nc.scalar.copy(fsg_sb[:, l, fb, r, :], fpp[:])

                # c_l = bi1 + sum_r b_lr @ Wi1_r   (feature-major [128,1] x2)
                b_sb = wld.tile([128, 2 * R, 4], f32)
                nc.gpsimd.memset(b_sb[:], 0.0)
                nc.sync.dma_start(
                    out=b_sb[:, :, 0:1],
                    in_=b0_d.rearrange("(c p) -> p c", p=128)[:, :, None],
                )
                for l in range(L - 1):
                    nc.sync.dma_start(
                        out=b_sb[:, :, l + 1:l + 2],
                        in_=bg_d[l].rearrange("(c p) -> p c", p=128)[:, :, None],
                    )
                b_sbr = wld.tile([128, 2 * R, 4], f32r)
                nc.vector.tensor_copy(b_sbr[:], b_sb[:])
                for m in range(2):
                    cp = wps.tile([128, 4], f32, tag="cp")
                    for ch in range(2 * R):
                        nc.tensor.matmul(
                            cp[:],
                            lhsT=wi1r[:, ch, 128 * m:128 * (m + 1)],
                            rhs=b_sbr[:, ch, :],
                            start=(ch == 0), stop=(ch == 2 * R - 1),
                        )
                    nc.scalar.activation(
                        c_sb[:, m, :], cp[:, 0:L], AF.Identity,
                        bias=bi1_fm[:, m:m + 1],
                    )

            # =======================================================
            # Stage 2: A-build (per-pair pipeline, no global barrier)
            # =======================================================
            with ExitStack() as ab:
                oh_pool = ab.enter_context(tc.tile_pool(name="oh", bufs=4))
                scat_sp = ab.enter_context(tc.tile_pool(name="scat_sb", bufs=1))
                stg_pool = ab.enter_context(tc.tile_pool(name="stg", bufs=1))
                ds_sp = ab.enter_context(tc.tile_pool(name="ds_sb", bufs=3))
                at1_pool = ab.enter_context(tc.tile_pool(name="at1", bufs=3))
                scat_pp = ab.enter_context(
                    tc.tile_pool(name="scat_ps", bufs=2, space="PSUM")
                )
                ds_pp = ab.enter_context(
                    tc.tile_pool(name="ds_ps", bufs=2, space="PSUM")
                )

                # all pairs' scatter results + degree rows stay resident; the
                # dis computation batches into one Ln + one Exp per HALF of
                # the pairs (pairs on partitions) — 4 ACT table loads total —
                # while halves pipeline against each other's one-hot builds
                NH = npair // 2
                scat_all = scat_sp.tile([128, npair, R, 60], bf16)
                deg_h = [
                    stg_pool.tile([NH, 2, R * 60], bf16, name=f"deg{h}",
                                  tag=f"deg{h}")
                    for h in range(2)
                ]
                dis_h = [
                    stg_pool.tile([NH, 2, R * 60], f32, name=f"dis{h}",
                                  tag=f"dis{h}")
                    for h in range(2)
                ]

                def emit_oh_scat(p):
                    ohs = []
                    for j in range(2):
                        g = 2 * p + j
                        oh = oh_pool.tile(
                            [128, R, 2, C, 62], bf16, tag="oh", name=f"oh{j}"
                        )
                        # one-hot compare, all operands 2-byte stride-1
                        nc.vector.tensor_tensor(
                            out=oh[:].rearrange(
                                "p r two c (s2 d2) -> p (r two c) s2 d2", s2=31
                            ),
                            in0=iota62[:].rearrange(
                                "p (s2 d2) -> p s2 d2", s2=31
                            )[:, None, :, :].to_broadcast([128, R * 2 * C, 31, 2]),
                            in1=eidx2[:, g, :, :, :, :].rearrange(
                                "p r two c d2 -> p (r two c) d2"
                            )[:, :, None, :].to_broadcast([128, R * 2 * C, 31, 2]),
                            op=OP.is_equal,
                        )
                        # x w on the src half (cols 0..59)
                        nc.vector.tensor_tensor(
                            out=oh[:, :, 0, :, 0:60].rearrange(
                                "p r c (s2 d2) -> p r c s2 d2", s2=30
                            ),
                            in0=oh[:, :, 0, :, 0:60].rearrange(
                                "p r c (s2 d2) -> p r c s2 d2", s2=30
                            ),
                            in1=ew2[:, g, :, :, :][:, :, :, None, :].to_broadcast(
                                [128, R, C, 30, 2]
                            ),
                            op=OP.mult,
                        )
                        # w column (degree row source)
                        nc.vector.tensor_copy(
                            out=oh[:, :, 0, :, 60:61],
                            in_=ew2[:, g, :, :, 0:1],
                        )
                        ohs.append(oh)

                    scat_ps = scat_pp.tile([128, R, 60], f32, tag="scat")
                    for r in range(R):
                        for c in range(C):
                            for j in range(2):
                                nc.tensor.matmul(
                                    scat_ps[64 * j:64 * j + 61, r, :],
                                    lhsT=ohs[j][:, r, 0, c, 0:61],
                                    rhs=ohs[j][:, r, 1, c, 0:60],
                                    start=(c == 0), stop=False,
                                    tile_position=(0, 64 * j),
                                )
                        for j in range(2):
                            nc.tensor.matmul(
                                scat_ps[64 * j:64 * j + 61, r, :],
                                lhsT=selfT[:],
                                rhs=i60[:],
                                start=False, stop=True,
                                tile_position=(0, 64 * j),
                            )
                    nc.scalar.copy(scat_all[:, p, :, :], scat_ps[:])
                    for j in range(2):
                        nc.sync.dma_start(
                            out=deg_h[p // NH][p % NH:p % NH + 1, j, :],
                            in_=scat_all[64 * j + 60:64 * j + 61, p, :, :],
                        )

                def emit_at(p):
                    # K=1 matmul lhsT must sit at partition 0 — restage row p
                    dis_p = ds_sp.tile([1, 2, R * 60], f32, tag="dis_p")
                    nc.gpsimd.dma_start(
                        out=dis_p[0:1],
                        in_=dis_h[p // NH][p % NH:p % NH + 1],
                    )
                    ds_ps = ds_pp.tile([60, 2, R, 60], f32, tag="ds")
                    for r in range(R):
                        for j in range(2):
                            row = dis_p[0:1, j, 60 * r:60 * (r + 1)]
                            nc.tensor.matmul(
                                ds_ps[0:60, j, r, :],
                                lhsT=row,
                                rhs=row,
                                start=True, stop=True,
                            )
                    # j=1 block staged at partition base 64 so every SBUF
                    # tensor_tensor sees equal input base partitions
                    ds_sb = ds_sp.tile([128, R, 60], bf16, tag="ds_sb")
                    nc.scalar.copy(ds_sb[0:60, :, :], ds_ps[0:60, 0, :, :])
                    nc.scalar.copy(ds_sb[64:124, :, :], ds_ps[0:60, 1, :, :])

                    # AT = (scat + I) * (dis x dis), block-diagonal pair tiles
                    nc.gpsimd.tensor_tensor(
                        out=AT_all[0:60, p, :, 0:60],
                        in0=scat_all[0:60, p, :, :],
                        in1=ds_sb[0:60, :, :],
                        op=OP.mult,
                    )
                    at1 = at1_pool.tile([128, R, 60], bf16, tag="at1")
                    nc.gpsimd.tensor_tensor(
                        out=at1[64:124, :, :],
                        in0=scat_all[64:124, p, :, :],
                        in1=ds_sb[64:124, :, :],
                        op=OP.mult,
                    )
                    nc.gpsimd.dma_start(
                        out=AT_all[60:120, p, :, 60:120], in_=at1[64:124, :, :]
                    )

                for h in range(2):
                    for p in range(NH * h, NH * (h + 1)):
                        emit_oh_scat(p)
                    # dis = exp(-0.5 ln deg) for this half's pairs
                    nc.scalar.activation(dis_h[h][:], deg_h[h][:], AF.Ln)
                    nc.scalar.activation(
                        dis_h[h][:], dis_h[h][:], AF.Exp, scale=-0.5
                    )
                    for p in range(NH * h, NH * (h + 1)):
                        emit_at(p)

            # wf1 resident in bf16 (60KB/partition), streamed through a small
            # f32 staging pool with gpsimd converts during the layer phase
            wf1P = top.enter_context(tc.tile_pool(name="wf1", bufs=1))
            wf1bf = wf1P.tile([128, NKC, D], bf16)
            wstg = top.enter_context(tc.tile_pool(name="wstg", bufs=3))

            # =======================================================
            # Stage 3: GCN layers
            # =======================================================
            with ExitStack() as lp:
                y_pp = lp.enter_context(
                    tc.tile_pool(name="y_ps", bufs=2, space="PSUM")
                )
                y_sp = lp.enter_context(tc.tile_pool(name="y_sb", bufs=4))
                ms_pp = lp.enter_context(
                    tc.tile_pool(name="ms_ps", bufs=1, space="PSUM")
                )
                hm_sp = lp.enter_context(tc.tile_pool(name="hmid", bufs=2))
                h_pp = lp.enter_context(
                    tc.tile_pool(name="h_ps", bufs=2, space="PSUM")
                )

                nconv = 0
                for l in range(L):
                    nk = 1 if l == 0 else 2
                    hT[l + 1] = hT_pool.tile(
                        [128, 2, nn], f32r, tag="hT", name=f"hT{l + 1}"
                    )
                    pdone = 0
                    while pdone < npair:
                        # one wf1 group per layer-group iteration (12 total):
                        # DMA on the idle sync queue, convert on idle gpsimd
                        if nconv < NKC // 10:
                            i = nconv
                            stg = wstg.tile([128, 10, D], f32, tag="wstg")
                            nc.sync.dma_start(
                                out=stg[:],
                                in_=wf1_d[
                                    1280 * i:1280 * (i + 1), :
                                ].rearrange("(c p) d -> p c d", p=128),
                            )
                            nc.gpsimd.tensor_copy(
                                out=wf1bf[:, 10 * i:10 * (i + 1), :],
                                in_=stg[:],
                            )
                            nconv += 1
                        gs = min(4, npair - pdone)  # pairs in this group
                        ms = [
                            ms_pp.tile([128, 120 * gs], f32, tag=f"ms{mt}",
                                       name=f"ms{mt}")
                            for mt in range(2)
                        ]
                        for pp in range(gs):
                            p = pdone + pp
                            y_ps = y_pp.tile([128, R * D], f32, tag="y")
                            for fs in range(2):
                                for kc in range(nk):
                                    if l == 0:
                                        rhs = fs0_sb[:].rearrange("p r d -> p (r d)")
                                    else:
                                        rhs = fsg_sb[:, l - 1, kc].rearrange(
                                            "p r d -> p (r d)"
                                        )
                                    nc.tensor.matmul(
                                        y_ps[0:120, 512 * fs:512 * (fs + 1)],
                                        lhsT=hT[l][
                                            :, kc, 120 * p:120 * (p + 1)
                                        ],
                                        rhs=rhs[:, 512 * fs:512 * (fs + 1)],
                                        start=(kc == 0), stop=(kc == nk - 1),
                                    )
                            y_sb = y_sp.tile([128, R, D], bf16, tag="ysb")
                            if pp % 2 == 0:
                                nc.vector.tensor_copy(
                                    y_sb[0:120].rearrange("p r d -> p (r d)"),
                                    y_ps[0:120, :],
                                )
                            else:
                                nc.scalar.copy(
                                    y_sb[0:120].rearrange("p r d -> p (r d)"),
                                    y_ps[0:120, :],
                                )
                            for mt in range(2):
                                for r in range(R):
                                    nc.tensor.matmul(
                                        ms[mt][:, 120 * pp:120 * (pp + 1)],
                                        lhsT=y_sb[0:120, r, 128 * mt:128 * (mt + 1)],
                                        rhs=AT_all[:, p, r, :],
                                        start=(r == 0), stop=(r == R - 1),
                                    )
                        hmid = hm_sp.tile([128, 2, 120 * gs], f32r, tag="hmid")
                        for mt in range(2):
                            nc.scalar.activation(
                                hmid[:, mt, :], ms[mt][:], AF.Relu,
                                bias=c_sb[:, mt, l:l + 1],
                            )
                        for mt2 in range(2):
                            hp = h_pp.tile([128, 120 * gs], f32, tag="hp")
                            for kc in range(2):
                                nc.tensor.matmul(
                                    hp[:],
                                    lhsT=wi2r[
                                        :, kc, 128 * mt2:128 * (mt2 + 1)
                                    ],
                                    rhs=hmid[:, kc, :],
                                    start=(kc == 0), stop=(kc == 1),
                                )
                            nc.scalar.activation(
                                hT[l + 1][:, mt2, 120 * pdone:120 * (pdone + gs)],
                                hp[:], AF.Identity, bias=bi2_fm[:, mt2:mt2 + 1],
                            )
                        pdone += gs

            # =======================================================
            # Stage 4: final FC (z1^T accumulation over native wf1 chunks)
            # =======================================================
            with ExitStack() as fp_:
                z_pp = fp_.enter_context(
                    tc.tile_pool(name="z_ps", bufs=1, space="PSUM")
                )
                z_sp = fp_.enter_context(tc.tile_pool(name="z_sb", bufs=1))

                # bf16 copy of h3 for the bf16 z1 matmuls
                h3b = z_sp.tile([128, 2, nn], bf16)
                nc.vector.tensor_copy(h3b[:], hT[L][:])
                z1_ps = z_pp.tile([32, D], f32, tag="z1", name="z1")
                for ch in range(NKC):
                    dc, n = ch & 1, ch >> 1
                    lhsT = h3b[:, dc, :].rearrange("p (g n) -> p n g", n=N)[:, n, :]
                    nc.tensor.matmul(
                        z1_ps[0:gc, :],
                        lhsT=lhsT,
                        rhs=wf1bf[:, ch, :],
                        start=(ch == 0), stop=False,
                    )
                # bias row: z1 += 1 (x) bf1
                nc.tensor.matmul(
                    z1_ps[0:gc, :],
                    lhsT=ones1[0:1, :],
                    rhs=bf1_row[0:1, :],
                    start=False, stop=True,
                )
                z1_sb = z_sp.tile([32, D], f32)
                nc.scalar.activation(z1_sb[0:gc, :], z1_ps[0:gc, :], AF.Relu)

                # transpose z1 -> feature-major [128, 2, gc]
                z1T_sb = z_sp.tile([128, 2, gc], bf16)
                for m in range(2):
                    ztp = z_pp.tile([128, 32], f32, tag="ztp", name=f"ztp{m}")
                    nc.tensor.transpose(
                        ztp[:, 0:gc], z1_sb[0:gc, 128 * m:128 * (m + 1)],
                        ident[0:gc, 0:gc],
                    )
                    nc.scalar.copy(z1T_sb[:, m, :], ztp[:, 0:gc])

                z2_ps = [
                    z_pp.tile([128, gc], f32, tag=f"z2_{mt}", name=f"z2_{mt}")
                    for mt in range(2)
                ]
                for mt in range(2):
                    for kc in range(2):
                        nc.tensor.matmul(
                            z2_ps[mt][:],
                            lhsT=wf2b[:, kc, 128 * mt:128 * (mt + 1)],
                            rhs=z1T_sb[:, kc, :],
                            start=(kc == 0), stop=(kc == 1),
                        )
                z2_sb = z_sp.tile([128, 2, gc], bf16)
                for mt in range(2):
                    nc.scalar.activation(
                        z2_sb[:, mt, :], z2_ps[mt][:], AF.Relu,
                        bias=bf2_fm[:, mt:mt + 1],
                    )
                z3_ps = z_pp.tile([2, gc], f32, tag="z3")
                for kc in range(2):
                    nc.tensor.matmul(
                        z3_ps[0:2, :],
                        lhsT=wf3b[:, kc, :],
                        rhs=z2_sb[:, kc, :],
                        start=(kc == 0), stop=(kc == 1),
                    )
                out_sb = z_sp.tile([2, gc], f32)
                nc.scalar.activation(
                    out_sb[0:2, :], z3_ps[0:2, :], AF.Identity,
                    bias=bf3_fm[0:2, 0:1],
                )
                nc.sync.dma_start(
                    out=out_d.rearrange("g j -> j g"), in_=out_sb[0:2, :]
                )

    nc.compile()
    return nc


def shard_inputs(inputs, gc=GC, ncores=NCORES):
    """Full inputs -> per-core in_maps (host-side layout only)."""
    x = np.ascontiguousarray(inputs["x"], dtype=np.float32)
    ei = np.ascontiguousarray(inputs["edge_index"], dtype=np.int32)
    ew = np.ascontiguousarray(inputs["edge_weight"], dtype=np.float32)
    shared = {
        "w0": np.ascontiguousarray(inputs["W_gcn0"], np.float32),
        "wg": np.ascontiguousarray(inputs["W_gcn"], np.float32),
        "b0": np.ascontiguousarray(inputs["b_gcn0"], np.float32).reshape(-1),
        "bg": np.ascontiguousarray(inputs["b_gcn"], np.float32).reshape(L - 1, -1),
        "wi1": np.ascontiguousarray(inputs["Wi1"], np.float32),
        "bi1": np.ascontiguousarray(inputs["bi1"], np.float32),
        "wi2": np.ascontiguousarray(inputs["Wi2"], np.float32),
        "bi2": np.ascontiguousarray(inputs["bi2"], np.float32),
        "wf1": np.ascontiguousarray(inputs["Wf1"], np.float32),
        "bf1": np.ascontiguousarray(inputs["bf1"], np.float32),
        "wf2": np.ascontiguousarray(inputs["Wf2"], np.float32),
        "bf2": np.ascontiguousarray(inputs["bf2"], np.float32),
        "wf3": np.ascontiguousarray(inputs["Wf3"], np.float32),
        "bf3": np.ascontiguousarray(inputs["bf3"], np.float32),
    }
    in_maps = []
    for c in range(ncores):
        s = slice(c * gc, (c + 1) * gc)
        m = dict(shared)
        m["x"] = np.ascontiguousarray(x[s].reshape(gc * N, F))
        m["ei"] = np.ascontiguousarray(ei[s].reshape(gc * R * 2, E))
        m["ew"] = np.ascontiguousarray(ew[s].reshape(gc * R, E))
        in_maps.append(m)
    return in_maps


def kernel(**inputs):
    from concourse import bass_utils

    if "nc" not in _CACHE:
        _CACHE["nc"] = _build(GC)
    nc = _CACHE["nc"]
    in_maps = shard_inputs(inputs)
    res = bass_utils.run_bass_kernel_spmd(
        nc, in_maps, core_ids=list(range(NCORES))
    )
    return np.concatenate([r["out"] for r in res.results], axis=0)
